# revision 3
# baseline (speedup 1.0000x reference)
"""Trainium2 Bass kernel: fused ConvLayersV2 (two stacked 3x3 VALID convs).

The two convs have no nonlinearity between them, so they compose exactly into
a single 5x5 VALID conv with effective weights W5[o,i,u,v] (computed host-side
in float64).  Data-parallel across 8 NeuronCores: one batch image per core.

Per-core layout:
  - Output rows are processed in blocks of 8 (Z in [0,64)); matmul M packs
    (row-phase c in [0,8)) x (out-channel o in [0,16)) = 128.
  - Contraction K packs (input-row offset q in [0,12)) x (in-channel i) = 36.
  - The 5 width taps (v) are 5 PSUM-accumulated matmuls with shifted rhs APs.
  - x lives in SBUF as one [36, 64, 512] tile: partition (q,i), free (Z, x),
    i.e. partition q*3+i holds input row 8Z+q of channel i (1.5x replicated).
  - Matmuls run in float32r (full-rate on TensorE for N>=256, ~fp32 accuracy
    via reduced-precision multiply with fp32 PSUM accumulation).
"""

import numpy as np

_CACHE = {}


def _build_bass():
    import concourse.bacc as bacc
    import concourse.bass as bass
    import concourse.tile as tile
    import concourse.mybir as mybir

    F32 = mybir.dt.float32
    F32R = mybir.dt.float32r

    nc = bacc.Bacc("TRN2", target_bir_lowering=False, debug=False)
    x_d = nc.dram_tensor("x", [3, 512, 512], F32R, kind="ExternalInput").ap()
    w_d = nc.dram_tensor("wt", [36, 640], F32R, kind="ExternalInput").ap()
    y_d = nc.dram_tensor("y", [16, 508, 508], F32, kind="ExternalOutput").ap()

    with tile.TileContext(nc) as tc:
        with (
            tc.tile_pool(name="wpool", bufs=1) as wpool,
            tc.tile_pool(name="xpool", bufs=1) as xpool,
            tc.tile_pool(name="opool", bufs=4) as opool,
            tc.tile_pool(name="psum", bufs=8, space=bass.MemorySpace.PSUM) as ppool,
        ):
            wt = wpool.tile([36, 640], F32R)
            nc.sync.dma_start(wt[:], w_d[:])

            # x tile: partition (q in 12) x (i in 3), free (Z in 64, x in 512)
            xt = xpool.tile([36, 64, 512], F32R)
            # partition-split view: (q, i, Z, x) — partition stride 3 per q
            xts = xt.rearrange("(q c) z x -> q c z x", c=3)
            # view of x as (r=row%8, c=channel, b=row//8, w)
            xx = x_d.rearrange("c (b r) w -> r c b w", r=8)
            for i in range(3):
                # q in [0,8): row 8Z+q = block Z, r=q  -> all 64 blocks
                for z0 in range(0, 64, 16):
                    nc.sync.dma_start(
                        xts[0:8, i, z0 : z0 + 16, :], xx[:, i, z0 : z0 + 16, :]
                    )
                # q in [8,12): row 8Z+q = block Z+1, r=q-8 -> blocks 1..63
                for z0 in range(0, 63, 21):
                    ch = min(21, 63 - z0)
                    nc.sync.dma_start(
                        xts[8:12, i, z0 : z0 + ch, :],
                        xx[0:4, i, 1 + z0 : 1 + z0 + ch, :],
                    )

            for z in range(64):
                tail = z == 63
                K = 24 if tail else 36  # tail: only q in [0,8) exists
                ps = ppool.tile([128, 508], F32, tag="ps")
                for v in range(5):
                    nc.tensor.matmul(
                        ps[:, :],
                        wt[0:K, v * 128 : (v + 1) * 128],
                        xt[0:K, z, v : v + 508],
                        start=(v == 0),
                        stop=(v == 4),
                    )
                P = 64 if tail else 128  # tail: only c in [0,4) valid
                ot = opool.tile([128, 508], F32, tag="ot")
                nc.vector.tensor_copy(ot[0:P, :], ps[0:P, :])
                nr = 4 if tail else 8
                yv = y_d[:, 8 * z : 8 * z + nr, :].transpose([1, 0, 2])
                nc.sync.dma_start(yv, ot[0:P, :])

    nc.compile()
    return nc


def _effective_weights(w1: np.ndarray, w2: np.ndarray) -> np.ndarray:
    """Compose conv1 (w1: [64,3,3,3]) and conv2 (w2: [16,64,3,3]) into the
    packed lhsT weight table wt[36, 640] (float32)."""
    w1 = np.asarray(w1, np.float64)
    w2 = np.asarray(w2, np.float64)
    W5 = np.zeros((16, 3, 5, 5), np.float64)
    for c in range(3):
        for d in range(3):
            W5[:, :, c : c + 3, d : d + 3] += np.einsum(
                "om,miab->oiab", w2[:, :, c, d], w1
            )
    # wt[q*3+i, v*128 + c*16 + o] = W5[o, i, q-c, v] (0 <= q-c < 5)
    wt = np.zeros((12, 3, 5, 8, 16), np.float64)
    for c in range(8):
        for u in range(5):
            q = c + u
            if q < 12:
                wt[q, :, :, c, :] = np.transpose(W5[:, :, u, :], (1, 2, 0))
    return np.ascontiguousarray(wt.reshape(36, 640).astype(np.float32))


def kernel(x: np.ndarray, w1: np.ndarray, w2: np.ndarray) -> np.ndarray:
    from concourse import bass_utils

    x = np.ascontiguousarray(np.asarray(x, np.float32))
    assert x.shape == (8, 3, 512, 512)
    wt = _effective_weights(w1, w2)

    if "nc" not in _CACHE:
        _CACHE["nc"] = _build_bass()
    nc = _CACHE["nc"]

    in_maps = [{"x": x[b], "wt": wt} for b in range(8)]
    res = bass_utils.run_bass_kernel_spmd(nc, in_maps, core_ids=list(range(8)))
    return np.stack([res.results[b]["y"] for b in range(8)]).astype(np.float32)


# revision 13
# speedup vs baseline: 1.1594x; 1.1594x over previous
"""Trainium2 Bass kernel: fused ConvLayersV2 (two stacked 3x3 VALID convs).

The two convs have no nonlinearity between them, so they compose exactly into
a single 5x5 VALID conv with effective weights W5[o,i,u,v] (computed host-side
in float64).  Data-parallel across 8 NeuronCores: one batch image per core.

Per-core layout (V2, parity-packed for PE row-group concurrency):
  - Output rows are processed in blocks of 8 (z in [0,64)); matmul M packs
    (row-phase c in [0,8)) x (out-channel o in [0,16)) = 128.
  - Contraction K packs (input-row offset q in [0,12)) x (in-channel i) = 36.
  - The 5 width taps (v) are 5 PSUM-accumulated matmuls with shifted rhs APs.
  - Even z windows live at SBUF partitions [0,36), odd z windows at [64,100)
    (PE row-groups {0,1} vs {2,3}), so even/odd matmul chains execute
    concurrently on the two halves of the systolic array.
  - x tile: [100, 32, 512]: partition (parity, q, i), free (zp, x); partition
    q*3+i holds row 16*zp+q, partition 64+q*3+i holds row 16*zp+8+q.
  - Matmuls run in float32r (full-rate on TensorE for N>=256, ~1e-4 rel err
    via reduced-precision multiply with fp32 PSUM accumulation).
"""

import numpy as np

_CACHE = {}


def _build_bass(reps: int = 1):
    import concourse.bacc as bacc
    import concourse.bass as bass
    import concourse.tile as tile
    import concourse.mybir as mybir

    F32 = mybir.dt.float32
    F32R = mybir.dt.float32r

    nc = bacc.Bacc("TRN2", target_bir_lowering=False, debug=False)
    x_d = nc.dram_tensor("x", [3, 512, 512], F32R, kind="ExternalInput").ap()
    w_d = nc.dram_tensor("wt", [36, 640], F32R, kind="ExternalInput").ap()
    y_d = nc.dram_tensor("y", [16, 508, 508], F32, kind="ExternalOutput").ap()

    with tile.TileContext(nc) as tc:
        with (
            tc.tile_pool(name="wpool", bufs=1) as wpool,
            tc.tile_pool(name="xpool", bufs=1) as xpool,
            tc.tile_pool(name="opool", bufs=6) as opool,
            tc.tile_pool(name="psum", bufs=8, space=bass.MemorySpace.PSUM) as ppool,
        ):
            for _rep in range(reps):
                _emit_body(nc, wpool, xpool, opool, ppool, x_d, w_d, y_d, F32, F32R)

    nc.compile()
    return nc


def _emit_body(nc, wpool, xpool, opool, ppool, x_d, w_d, y_d, F32, F32R):
    if True:
        if True:
            # weights duplicated at partition 0 (even z) and 64 (odd z)
            wt = wpool.tile([100, 640], F32R)
            nc.scalar.dma_start(wt[0:36, :], w_d[:])
            nc.gpsimd.dma_start(wt[64:100, :], w_d[:])

            # x tile: [100, 32, 512]; (q,i) split views for even/odd halves
            xt = xpool.tile([100, 32, 512], F32R)
            ev = xt[0:36].rearrange("(q c) z x -> q c z x", c=3)
            od = xt[64:100].rearrange("(q c) z x -> q c z x", c=3)
            # x as (r=row%16, c=channel, zp=row//16, w)
            xxp = x_d.rearrange("c (zp r) w -> r c zp w", r=16)
            # zp chunk ladder: small first chunk so matmuls start early; the
            # first chunk's DMAs are spread across all three DGE paths so
            # descriptor generation doesn't serialize the pipeline start
            z0 = 0
            for ci, CH in enumerate((2, 6, 8, 8, 8)):
                zl = slice(z0, z0 + CH)
                for i in range(3):
                    if ci == 0:
                        e1, e2, e3 = (
                            (nc.sync, nc.scalar, nc.gpsimd),
                            (nc.scalar, nc.gpsimd, nc.sync),
                            (nc.gpsimd, nc.sync, nc.scalar),
                        )[i]
                    else:
                        e1 = e2 = e3 = nc.sync
                    # even window q in [0,12): rows 16zp+q
                    e1.dma_start(ev[0:12, i, zl, :], xxp[0:12, i, zl, :])
                    # odd window q in [0,12): rows 16zp+8+q
                    e2.dma_start(od[0:8, i, zl, :], xxp[8:16, i, zl, :])
                    z1 = min(z0 + CH, 31)
                    if z1 > z0:
                        e3.dma_start(
                            od[8:12, i, z0:z1, :], xxp[0:4, i, z0 + 1 : z1 + 1, :]
                        )
                z0 += CH

            for zp in range(32):
                tail = zp == 31  # odd z = 63
                pse = ppool.tile([128, 508], F32, tag="ps")
                pso = ppool.tile([128, 508], F32, tag="ps")
                Ko = 24 if tail else 36
                for v in range(5):
                    nc.tensor.matmul(
                        pse[:, :],
                        wt[0:36, v * 128 : (v + 1) * 128],
                        xt[0:36, zp, v : v + 508],
                        start=(v == 0),
                        stop=(v == 4),
                    )
                    nc.tensor.matmul(
                        pso[:, :],
                        wt[64 : 64 + Ko, v * 128 : (v + 1) * 128],
                        xt[64 : 64 + Ko, zp, v : v + 508],
                        start=(v == 0),
                        stop=(v == 4),
                    )
                for par, ps in ((0, pse), (1, pso)):
                    z = 2 * zp + par
                    P = 64 if z == 63 else 128  # tail: only c in [0,4) valid
                    ot = opool.tile([128, 508], F32, tag="ot")
                    if par == 0:
                        nc.vector.tensor_copy(ot[0:P, :], ps[0:P, :])
                    else:
                        nc.scalar.copy(ot[0:P, :], ps[0:P, :])
                    nr = 4 if z == 63 else 8
                    yv = y_d[:, 8 * z : 8 * z + nr, :].transpose([1, 0, 2])
                    # spread output DMAs: SWDGE (gpsimd) runs its descriptor
                    # generation on the otherwise-idle Pool engine, in
                    # parallel with the HWDGE rings on SP/ACT
                    eng = (nc.scalar, nc.gpsimd)[z % 2]
                    eng.dma_start(yv, ot[0:P, :])


def _effective_weights(w1: np.ndarray, w2: np.ndarray) -> np.ndarray:
    """Compose conv1 (w1: [64,3,3,3]) and conv2 (w2: [16,64,3,3]) into the
    packed lhsT weight table wt[36, 640] (float32)."""
    w1 = np.asarray(w1, np.float64)
    w2 = np.asarray(w2, np.float64)
    W5 = np.zeros((16, 3, 5, 5), np.float64)
    for c in range(3):
        for d in range(3):
            W5[:, :, c : c + 3, d : d + 3] += np.einsum(
                "om,miab->oiab", w2[:, :, c, d], w1
            )
    # wt[q*3+i, v*128 + c*16 + o] = W5[o, i, q-c, v] (0 <= q-c < 5)
    wt = np.zeros((12, 3, 5, 8, 16), np.float64)
    for c in range(8):
        for u in range(5):
            q = c + u
            if q < 12:
                wt[q, :, :, c, :] = np.transpose(W5[:, :, u, :], (1, 2, 0))
    return np.ascontiguousarray(wt.reshape(36, 640).astype(np.float32))


def kernel(x: np.ndarray, w1: np.ndarray, w2: np.ndarray) -> np.ndarray:
    from concourse import bass_utils

    x = np.ascontiguousarray(np.asarray(x, np.float32))
    assert x.shape == (8, 3, 512, 512)
    wt = _effective_weights(w1, w2)

    if "nc" not in _CACHE:
        _CACHE["nc"] = _build_bass()
    nc = _CACHE["nc"]

    in_maps = [{"x": x[b], "wt": wt} for b in range(8)]
    res = bass_utils.run_bass_kernel_spmd(nc, in_maps, core_ids=list(range(8)))
    return np.stack([res.results[b]["y"] for b in range(8)]).astype(np.float32)
